# revision 6
# baseline (speedup 1.0000x reference)
"""Bass/Trainium2 kernel v5b (bf16 RS payload, dedicated staging tile) for nn_BaselineAttention (dense transformer block).

Sharding: data-parallel over batch (2 groups of 4 cores) x tensor-parallel
over heads (4 heads per core). Transposed dataflow ([feature, token]);
softmax denominator via ones-column in V; denominator broadcast via a 4x128
selection matmul; LN1 stats via gpsimd partition_all_reduce packed into one
[1,4096] AllReduce; broadcasts via gpsimd partition_broadcast; LN2 fused on a
single [128,4,1024] tile with middle-axis broadcast APs. Minimizes
instruction count (dispatch-bound runtime) and scalar-engine usage
(superlinear cost at scale).
"""

import ml_dtypes
import numpy as np

import bass_rust
import concourse.bacc as bacc
import concourse.mybir as mybir
import concourse.tile as tile
from concourse.bass_utils import run_bass_kernel_spmd

F32 = mybir.dt.float32
F32R = mybir.dt.float32r
BF16 = mybir.dt.bfloat16
AF = mybir.ActivationFunctionType
OP = mybir.AluOpType
AX = mybir.AxisListType
RED = bass_rust.ReduceOp

B, S, D, H = 2, 2048, 1024, 16
EPS = 1e-3
SCALE = 0.125            # 1/sqrt(D/H)
GROUPS = [[0, 1, 2, 3], [4, 5, 6, 7]]

_BUILD_CACHE = {}


def _build(n_reps=1):
    if n_reps in _BUILD_CACHE:
        return _BUILD_CACHE[n_reps]

    nc = bacc.Bacc("TRN2", target_bir_lowering=False, debug=False, num_devices=8)

    xt_d = nc.dram_tensor("xt", [128, 8, S], F32R, kind="ExternalInput").ap()
    wq_d = nc.dram_tensor("wq", [128, 8, 128], F32R, kind="ExternalInput").ap()
    wk_d = nc.dram_tensor("wk", [128, 8, 128], F32R, kind="ExternalInput").ap()
    wv_d = nc.dram_tensor("wv", [128, 8, 256], F32R, kind="ExternalInput").ap()
    wfc_d = nc.dram_tensor("wfc", [128, 2, D], F32R, kind="ExternalInput").ap()
    onesv_d = nc.dram_tensor("onesv", [128, 64], BF16, kind="ExternalInput").ap()
    ident_d = nc.dram_tensor("ident", [128, 128], BF16, kind="ExternalInput").ap()
    sel4_d = nc.dram_tensor("sel4", [4, 2, 128], F32R, kind="ExternalInput").ap()
    bq_d = nc.dram_tensor("bq", [128, 1], F32, kind="ExternalInput").ap()
    bk_d = nc.dram_tensor("bk", [128, 1], F32, kind="ExternalInput").ap()
    bv_d = nc.dram_tensor("bv", [128, 2], F32, kind="ExternalInput").ap()
    g1_d = nc.dram_tensor("g1", [128, 2], F32, kind="ExternalInput").ap()
    b1_d = nc.dram_tensor("b1", [128, 2], F32, kind="ExternalInput").ap()
    g2row_d = nc.dram_tensor("g2row", [1, D], F32, kind="ExternalInput").ap()
    b2row_d = nc.dram_tensor("b2row", [1, D], F32, kind="ExternalInput").ap()
    bfcrow_d = nc.dram_tensor("bfcrow", [1, D], F32, kind="ExternalInput").ap()
    out_d = nc.dram_tensor("out", [4, 128, D], F32, kind="ExternalOutput").ap()

    with (
        tile.TileContext(nc) as tc,
        tc.tile_pool(name="sb", bufs=1) as sb,
        tc.tile_pool(name="ps", bufs=1, space="PSUM") as ps,
        tc.tile_pool(name="dr", bufs=1, space="DRAM") as dr,
    ):
        ident = sb.tile([128, 128], BF16)
        sel4 = sb.tile([4, 2, 128], F32R)
        wq = sb.tile([128, 8, 128], F32R)
        wk = sb.tile([128, 8, 128], F32R)
        wv = sb.tile([128, 8, 256], F32R)
        wfc = sb.tile([128, 2, D], F32R)
        bq = sb.tile([128, 1], F32)
        bk = sb.tile([128, 1], F32)
        bv = sb.tile([128, 2], F32)
        g1 = sb.tile([128, 2], F32)
        b1 = sb.tile([128, 2], F32)
        for t, d in [(ident, ident_d), (sel4, sel4_d),
                     (wq, wq_d), (wk, wk_d), (wv, wv_d), (wfc, wfc_d),
                     (bq, bq_d), (bk, bk_d), (bv, bv_d), (g1, g1_d), (b1, b1_d)]:
            nc.sync.dma_start(t[:], d[:])

        # broadcast gamma2 / beta2 / bfc rows to [128, D] once
        g2bc = sb.tile([128, D], F32)
        b2bc = sb.tile([128, D], F32)
        fcbc = sb.tile([128, D], F32)
        for i, (d, dst) in enumerate([(g2row_d, g2bc), (b2row_d, b2bc),
                                      (bfcrow_d, fcbc)]):
            rowt = sb.tile([1, D], F32, tag="rowinit", bufs=1, name=f"ri{i}")
            nc.sync.dma_start(rowt[:], d[:])
            nc.gpsimd.partition_broadcast(dst[:], rowt[:])

        # persistent state
        vnat = sb.tile([128, 16, 260], BF16)   # V natural + ones cols
        nc.sync.dma_start(
            vnat[:].rearrange("p t (h x) -> p (t h) x", h=4)[:, :, 64:65],
            onesv_d[:].unsqueeze(2),
        )
        qt_sb = sb.tile([128, S], BF16)        # [qk-feat, tok]
        kt_sb = sb.tile([128, S], BF16)
        ysb_t = sb.tile([128, 2, S], F32R)     # [vfeat-local, jj, tok]
        ysb = ysb_t[:]
        ut_sb = ysb_t[:].bitcast(F32)          # f32 view for DVE reads
        ysb2_t = sb.tile([128, 2, S], F32R)    # LN1 output (kept separate so
        ysb2 = ysb2_t[:]                       # hw-looped reps stay idempotent)
        dall = sb.tile([4, S], F32R)           # per-head denominators
        pP8 = sb.tile([128, 4, D], BF16)       # LN2 staging (dedicated: no
                                               # ring aliasing with DVE reads)

        stats_in = dr.tile([1, 2 * S], F32)    # [sum | sumsq]
        stats_out = dr.tile([1, 2 * S], F32)
        rs_in = dr.tile([S, D], BF16)
        rs_out = dr.tile([512, D], BF16)

        from contextlib import nullcontext

        def loop():
            return tc.For_i(0, n_reps) if n_reps > 1 else nullcontext()

        rep = 0
        with loop():
            # ---------------- P1: projections ----------------
            for half in range(4):
                xs = sb.tile([128, 8, 512], F32R, tag="xs", bufs=1,
                             name=f"r{rep}xs{half}")
                nc.sync.dma_start(xs[:], xt_d[:, :, 512 * half:512 * half + 512])
                for pname, w_t, mcol, bias, dst in [
                    ("q", wq, None, bq[:], qt_sb[:]),
                    ("k", wk, None, bk[:], kt_sb[:]),
                    ("v0", wv, slice(0, 128), bv[:, 0:1], None),
                    ("v1", wv, slice(128, 256), bv[:, 1:2], None),
                ]:
                    p_t = ps.tile([128, 512], F32,
                                  tag="tagA" if pname in ("q", "v0") else "tagB",
                                  bufs=1, name=f"r{rep}p{pname}{half}")
                    for kc in range(8):
                        nc.tensor.matmul(
                            p_t[:],
                            w_t[:, kc, :] if mcol is None else w_t[:, kc, mcol],
                            xs[:, kc, :],
                            start=(kc == 0), stop=(kc == 7))
                    if pname in ("q", "k"):
                        nc.vector.tensor_scalar(
                            dst[:, 512 * half:512 * half + 512], p_t[:],
                            bias, None, OP.add)
                    else:
                        jj = 0 if pname == "v0" else 1
                        vt_st = sb.tile([128, 512], BF16, tag="vtst", bufs=1,
                                        name=f"r{rep}vt{jj}{half}")
                        nc.vector.tensor_scalar(vt_st[:], p_t[:], bias, None, OP.add)
                        t_ps = ps.tile([128, 512], BF16,
                                       tag="tagA" if pname == "v0" else "tagB",
                                       bufs=1, name=f"r{rep}t{jj}{half}")
                        for blk in range(4):
                            nc.tensor.transpose(
                                t_ps[:, 128 * blk:128 * blk + 128],
                                vt_st[:, 128 * blk:128 * blk + 128], ident[:])
                        nc.vector.tensor_copy(
                            vnat[:, 4 * half:4 * half + 4, :]
                            .rearrange("p t (h x) -> p t h x", h=4)
                            [:, :, 2 * jj:2 * jj + 2, 0:64],
                            t_ps[:].rearrange("p (t h x) -> p t h x", t=4, h=2),
                        )

            # ---------------- P2: attention ----------------
            for hl in range(4):
                s_ps = ps.tile([128, 2048], F32, tag="tagA", bufs=1,
                               name=f"r{rep}s{hl}")
                u_ps = ps.tile([65, 2048], F32, tag="tagB", bufs=1,
                               name=f"r{rep}u{hl}")
                e_t = sb.tile([128, 2048], BF16, tag="e", bufs=1, name=f"r{rep}e{hl}")
                for kc in range(16):
                    for u in range(4):
                        nc.tensor.matmul(
                            s_ps[:, 512 * u:512 * u + 512],
                            kt_sb[32 * hl:32 * hl + 32, 128 * kc:128 * kc + 128],
                            qt_sb[32 * hl:32 * hl + 32, 512 * u:512 * u + 512],
                            tile_position=(32 * hl, 0), start=True, stop=True)
                    nc.scalar.activation(e_t[:], s_ps[:], AF.Exp, scale=SCALE)
                    for u in range(4):
                        nc.tensor.matmul(
                            u_ps[:, 512 * u:512 * u + 512],
                            vnat[:, kc, 65 * hl:65 * hl + 65],
                            e_t[:, 512 * u:512 * u + 512],
                            start=(kc == 0), stop=(kc == 15))
                u_st = sb.tile([65, S], F32R, tag="ust", bufs=1, name=f"r{rep}ust{hl}")
                nc.vector.tensor_copy(u_st[:], u_ps[:])
                nc.sync.dma_start(
                    ysb[64 * (hl % 2):64 * (hl % 2) + 64, hl // 2, :], u_st[0:64, :])
                nc.sync.dma_start(dall[hl:hl + 1, :], u_st[64:65, :])

            # ---------------- P3: divide, LN1 ----------------
            for jj in range(2):
                db_ps = ps.tile([128, 2048], F32,
                                tag="tagA" if jj == 0 else "tagB",
                                bufs=1, name=f"r{rep}db{jj}")
                for u in range(4):
                    nc.tensor.matmul(
                        db_ps[:, 512 * u:512 * u + 512], sel4[:, jj, :],
                        dall[:, 512 * u:512 * u + 512], start=True, stop=True)
                rec = sb.tile([128, 2048], F32, tag="scratch", bufs=2,
                              name=f"r{rep}rec{jj}")
                nc.vector.reciprocal_approx_fast(rec[:], db_ps[:])
                nc.vector.tensor_tensor(ysb[:, jj, :], ut_sb[:, jj, :], rec[:],
                                        OP.mult)
            ysq = sb.tile([128, 2, 2048], F32, tag="big2", bufs=1,
                          name=f"r{rep}ysq")
            nc.vector.tensor_tensor(ysq[:], ut_sb[:], ut_sb[:], OP.mult)
            ar1 = sb.tile([128, 4096], F32, tag="scratch", bufs=2, name=f"r{rep}ar1")
            nc.gpsimd.partition_all_reduce(ar1[:], ut_sb[:], channels=128,
                                           reduce_op=RED.add)
            srow = sb.tile([1, 4096], F32, tag="row", bufs=1, name=f"r{rep}srow")
            nc.vector.tensor_tensor(srow[0:1, 0:2048], ar1[0:1, 0:2048],
                                    ar1[0:1, 2048:4096], OP.add)
            nc.gpsimd.partition_all_reduce(
                ar1[:], ysq[:].rearrange("p j t -> p (j t)"), channels=128,
                reduce_op=RED.add)
            nc.vector.tensor_tensor(srow[0:1, 2048:4096], ar1[0:1, 0:2048],
                                    ar1[0:1, 2048:4096], OP.add)
            nc.sync.dma_start(stats_in[:], srow[:])
        for _r in range(n_reps):
            nc.gpsimd.collective_compute(
                "AllReduce", OP.add, replica_groups=GROUPS,
                ins=[stats_in[:]], outs=[stats_out[:]])
        with loop():
            strow = sb.tile([1, 4096], F32, tag="row", bufs=1, name=f"r{rep}strow")
            nc.sync.dma_start(strow[:], stats_out[:])
            nc.vector.tensor_scalar(strow[:], strow[:], 1.0 / D, None, OP.mult)
            r_row = sb.tile([1, 2048], F32, tag="row2", bufs=1, name=f"r{rep}rrow")
            nc.vector.tensor_tensor(r_row[:], strow[0:1, 0:2048],
                                    strow[0:1, 0:2048], OP.mult)
            nc.vector.tensor_tensor(r_row[:], strow[0:1, 2048:4096], r_row[:],
                                    OP.subtract)
            nc.vector.tensor_scalar(r_row[:], r_row[:], EPS, None, OP.add)
            nc.vector.reciprocal_approx_fast(r_row[:], r_row[:])
            nc.scalar.activation(r_row[:], r_row[:], AF.Sqrt)
            nc.vector.tensor_tensor(strow[0:1, 2048:4096], strow[0:1, 0:2048],
                                    r_row[:], OP.mult)
            rbc = sb.tile([128, 2048], F32, tag="ust", bufs=1, name=f"r{rep}rbc")
            mrbc = sb.tile([128, 2048], F32, tag="big2", bufs=1, name=f"r{rep}mrbc")
            nc.gpsimd.partition_broadcast(rbc[:], r_row[:])
            nc.gpsimd.partition_broadcast(mrbc[:], strow[0:1, 2048:4096])
            ut2 = ysb2_t[:].bitcast(F32)
            for jj in range(2):
                # y' = (u*g1)*r - (mr*g1 - b1)
                nc.vector.scalar_tensor_tensor(
                    ysb2[:, jj, :], ut_sb[:, jj, :], g1[:, jj:jj + 1], rbc[:],
                    OP.mult, OP.mult)
                cjj = sb.tile([128, 2048], F32, tag="scratch", bufs=2,
                              name=f"r{rep}c{jj}")
                nc.vector.tensor_scalar(cjj[:], mrbc[:], g1[:, jj:jj + 1],
                                        b1[:, jj:jj + 1], OP.mult, OP.subtract)
                nc.vector.tensor_tensor(ysb2[:, jj, :], ut2[:, jj, :], cjj[:],
                                        OP.subtract)

            # ---------------- fc + RS ----------------
            for pair in range(8):       # 2 token-chunks of 128 per psum tile
                fc_ps = ps.tile([128, 2048], F32,
                                tag="tagA" if pair % 2 == 0 else "tagB",
                                bufs=1, name=f"r{rep}fc{pair}")
                for half in range(2):
                    tok = slice(256 * pair + 128 * half, 256 * pair + 128 * half + 128)
                    for jj in range(2):
                        for nch in range(2):
                            nc.tensor.matmul(
                                fc_ps[:, 1024 * half + 512 * nch:
                                      1024 * half + 512 * nch + 512],
                                ysb2[:, jj, tok],
                                wfc[:, jj, 512 * nch:512 * nch + 512],
                                start=(jj == 0), stop=(jj == 1))
                p_st = sb.tile([128, 2048], BF16, tag="e", bufs=1,
                               name=f"r{rep}pst{pair}")
                nc.vector.tensor_copy(p_st[:], fc_ps[:])
                nc.sync.dma_start(
                    rs_in[256 * pair:256 * pair + 256, :]
                    .rearrange("(t p) n -> p t n", t=2),
                    p_st[:].rearrange("p (t n) -> p t n", t=2))
        for _r in range(n_reps):
            nc.gpsimd.collective_compute(
                "ReduceScatter", OP.add, replica_groups=GROUPS,
                ins=[rs_in[:]], outs=[rs_out[:]])
        with loop():
            # ---------------- LN2 (one [128,4,1024] tile) ----------------
            nc.sync.dma_start(pP8[:], rs_out[:].rearrange("(c p) n -> p c n", c=4))
            pP = sb.tile([128, 4, D], F32, tag="big2", bufs=1, name=f"r{rep}pP")
            nc.vector.tensor_tensor(
                pP[:], pP8[:], fcbc[:].unsqueeze(1).to_broadcast((128, 4, D)),
                OP.add)
            s8 = sb.tile([128, 12], F32, tag="s2", bufs=2, name=f"r{rep}s8")
            nc.vector.tensor_reduce(s8[:, 0:4], pP[:], AX.X, OP.add)
            sq = sb.tile([128, 4, D], F32, tag="scratch", bufs=2, name=f"r{rep}sq")
            nc.vector.tensor_tensor(sq[:], pP[:], pP[:], OP.mult)
            nc.vector.tensor_reduce(s8[:, 4:8], sq[:], AX.X, OP.add)
            nc.vector.tensor_scalar(s8[:, 0:8], s8[:, 0:8], 1.0 / D, None, OP.mult)
            nc.vector.tensor_tensor(s8[:, 8:12], s8[:, 0:4], s8[:, 0:4], OP.mult)
            nc.vector.tensor_tensor(s8[:, 8:12], s8[:, 4:8], s8[:, 8:12],
                                    OP.subtract)
            nc.vector.tensor_scalar(s8[:, 8:12], s8[:, 8:12], EPS, None, OP.add)
            nc.vector.reciprocal_approx_fast(s8[:, 8:12], s8[:, 8:12])
            nc.scalar.activation(s8[:, 8:12], s8[:, 8:12], AF.Sqrt)
            for c in range(4):
                nc.vector.tensor_scalar(pP[:, c, :], pP[:, c, :],
                                        s8[:, c:c + 1], s8[:, 8 + c:9 + c],
                                        OP.subtract, OP.mult)
            nc.vector.tensor_tensor(
                pP[:], pP[:], g2bc[:].unsqueeze(1).to_broadcast((128, 4, D)),
                OP.mult)
            nc.vector.tensor_tensor(
                pP[:], pP[:], b2bc[:].unsqueeze(1).to_broadcast((128, 4, D)),
                OP.add)
            nc.sync.dma_start(out_d[:].rearrange("c p n -> p c n"), pP[:])

    nc.compile()
    _BUILD_CACHE[n_reps] = nc
    return nc


def make_in_maps(x, Wq, bq, Wk, bk, Wv, bv, gamma1, beta1, Wfc, bfc, gamma2, beta2):
    x = np.asarray(x, np.float32)
    in_maps = []
    onesv = np.ones((128, 64), ml_dtypes.bfloat16)
    ident = np.eye(128, dtype=ml_dtypes.bfloat16)
    sel4 = np.zeros((4, 2, 128), np.float32)
    for jj in range(2):
        for p in range(128):
            sel4[2 * jj + p // 64, jj, p] = 1.0
    Wq, Wk, Wv, Wfc = (np.asarray(a, np.float32) for a in (Wq, Wk, Wv, Wfc))
    for c in range(8):
        g, r = c // 4, c % 4
        xt = np.ascontiguousarray(
            x[g].T.reshape(8, 128, S).transpose(1, 0, 2))          # [128, 8, S]
        wq_c = np.ascontiguousarray(
            Wq[:, 128 * r:128 * r + 128].reshape(8, 128, 128).transpose(1, 0, 2))
        wk_c = np.ascontiguousarray(
            Wk[:, 128 * r:128 * r + 128].reshape(8, 128, 128).transpose(1, 0, 2))
        wv_c = np.ascontiguousarray(
            Wv[:, 256 * r:256 * r + 256].reshape(8, 128, 256).transpose(1, 0, 2))
        wfc_c = np.ascontiguousarray(
            Wfc[256 * r:256 * r + 256, :].reshape(2, 128, D).transpose(1, 0, 2))
        in_maps.append({
            "xt": xt, "wq": wq_c, "wk": wk_c, "wv": wv_c, "wfc": wfc_c,
            "onesv": onesv, "ident": ident, "sel4": sel4,
            "bq": np.asarray(bq, np.float32)[128 * r:128 * r + 128, None],
            "bk": np.asarray(bk, np.float32)[128 * r:128 * r + 128, None],
            "bv": np.asarray(bv, np.float32)[256 * r:256 * r + 256]
                 .reshape(2, 128).T.copy(),
            "g1": np.asarray(gamma1, np.float32)[256 * r:256 * r + 256]
                 .reshape(2, 128).T.copy(),
            "b1": np.asarray(beta1, np.float32)[256 * r:256 * r + 256]
                 .reshape(2, 128).T.copy(),
            "g2row": np.asarray(gamma2, np.float32)[None, :].copy(),
            "b2row": np.asarray(beta2, np.float32)[None, :].copy(),
            "bfcrow": np.asarray(bfc, np.float32)[None, :].copy(),
        })
    return in_maps


def assemble(results):
    out = np.empty((B, S, D), np.float32)
    for c in range(8):
        g, r = c // 4, c % 4
        o = results[c]["out"]                   # [4, 128, D] = slab r of batch g
        for ts in range(4):
            out[g, 512 * r + 128 * ts:512 * r + 128 * ts + 128, :] = o[ts]
    return out


def kernel(**inputs):
    nc = _build()
    in_maps = make_in_maps(**{k: np.asarray(v) for k, v in inputs.items()})
    res = run_bass_kernel_spmd(nc, in_maps, list(range(8)))
    return assemble(res.results)
